# revision 34
# baseline (speedup 1.0000x reference)
import sys
sys.path.insert(0, "/opt/trn_rl_repo")
import numpy as np
import ml_dtypes
import concourse.bacc as bacc
import concourse.bass as bass
import concourse.mybir as mybir
import concourse.tile as tile
from concourse.bass import ds, ts
from concourse.bass_utils import run_bass_kernel_spmd

BF = ml_dtypes.bfloat16
P = 128
NT = 577          # tokens
NPAD = 580        # tokens padded to 4*145
NG = 145          # token groups of 4 (for channel-mix transposes)
D = 768
H = 16
HD = 48
KO = 7            # 896 = 7*128 contraction tiles (768 dims + bias row + pad)
NBLK = [(0, 128), (128, 128), (256, 128), (384, 128), (512, 65)]
SCALE = HD ** -0.5

_cache = {}


def _build(nc):
    bf = mybir.dt.bfloat16
    f32 = mybir.dt.float32
    Exp = mybir.ActivationFunctionType.Exp
    Ln = mybir.ActivationFunctionType.Ln

    x_d = nc.dram_tensor("xT", [896, NT], bf, kind="ExternalInput")
    xf_d = nc.dram_tensor("xfT", [896, NT], bf, kind="ExternalInput")
    wqk_d = nc.dram_tensor("wqkT", [896, 2048], bf, kind="ExternalInput")
    wv_d = nc.dram_tensor("wvT", [896, 16 * 49], bf, kind="ExternalInput")
    wb_d = nc.dram_tensor("wblk", [P, 64], bf, kind="ExternalInput")
    id_d = nc.dram_tensor("idn", [P, P], bf, kind="ExternalInput")
    pw_d = nc.dram_tensor("pwT", [896, D], bf, kind="ExternalInput")
    out_d = nc.dram_tensor("outT", [D, NT], f32, kind="ExternalOutput")

    with tile.TileContext(nc) as tc:
        with tc.tile_pool(name="wqk", bufs=1) as wqkp, \
             tc.tile_pool(name="big", bufs=1) as big, \
             tc.tile_pool(name="xw", bufs=2) as xw, \
             tc.tile_pool(name="qk", bufs=1) as qk, \
             tc.tile_pool(name="sc", bufs=3) as sc, \
             tc.tile_pool(name="rsp", bufs=6) as rsp, \
             tc.tile_pool(name="psB", bufs=2, space="PSUM") as psB, \
             tc.tile_pool(name="psT", bufs=2, space="PSUM") as psT, \
             tc.tile_pool(name="psM", bufs=2, space="PSUM") as psM:

            # ---- load weights & inputs (q-half first so qkv can start early) ----
            wqk = wqkp.tile([P, KO, 2048], bf, tag="wqk")
            x_sb = xw.tile([P, KO, NPAD], bf, tag="xw")
            xf_sb = xw.tile([P, KO, NPAD], bf, tag="xw")
            nc.sync.dma_start(x_sb[:, :, :NT], x_d.rearrange("(ko p) m -> p ko m", p=P))
            nc.sync.dma_start(wqk[:, :, 0:1024], wqk_d[:, 0:1024].rearrange("(ko p) m -> p ko m", p=P))
            nc.sync.dma_start(wqk[:, :, 1024:2048], wqk_d[:, 1024:2048].rearrange("(ko p) m -> p ko m", p=P))
            nc.sync.dma_start(xf_sb[:, :, :NT], xf_d.rearrange("(ko p) m -> p ko m", p=P))
            wblk = qk.tile([P, 64], bf)
            idn = qk.tile([P, P], bf)
            nc.sync.dma_start(wblk[:], wb_d[:])
            nc.sync.dma_start(idn[:], id_d[:])
            wv = big.tile([P, KO, 784], bf, tag="E2T")  # slot reused by E2T later
            nc.sync.dma_start(wv[:], wv_d.rearrange("(ko p) m -> p ko m", p=P))

            # ---- qkv + pass 1 interleaved ----
            # qt/kt rows per head h: partitions 64*(h%2) .. +47 at t=h//2;
            # row 64*(h%2)+48 is the augmentation slot (ones for k, -ln z for q).
            # Pass-1 exp/reduce for the x-branch overlaps the PE-only qkv of the
            # freq branch; pass-1 of the freq branch overlaps the v matmuls.
            qt = qk.tile([P, 8, NT], bf)
            kt = qk.tile([P, 8, NT], bf)
            qft = qk.tile([P, 8, NT], bf)
            kft = qk.tile([P, 8, NT], bf)
            v_sb = qk.tile([P, 5, 784], bf)
            zt = qk.tile([P, 5, 32], f32)

            def emit_qkv(dst, wt_, src, t):
                pp = psB.tile([P, 784], f32, tag="big", name="pp")
                for ko in range(KO):
                    for o0, w in ((0, 512), (512, 65)):
                        nc.tensor.matmul(pp[:, o0:o0 + w], wqk[:, ko, ds(1024 * wt_ + 128 * t, P)],
                                         src[:, ko, ds(o0, w)],
                                         start=(ko == 0), stop=(ko == KO - 1))
                nc.vector.tensor_copy(dst[:, t, :NT], pp[:, :NT])

            def emit_pass1(c):
                qs, ks = (qt, kt) if c < 16 else (qft, kft)
                h = c % 16
                t, off = h // 2, 64 * (h % 2)
                for bi, (n0, nlen) in enumerate(NBLK):
                    sp = psB.tile([P, 784], f32, tag="big", name="sp")
                    for o0, w in ((0, 512), (512, 65)):
                        nc.tensor.matmul(sp[:nlen, o0:o0 + w], qs[off:off + 48, t, ds(n0, nlen)],
                                         ks[off:off + 48, t, ds(o0, w)], start=True, stop=True)
                    scr = sc.tile([P, NPAD], bf, tag="scr", name="scr")
                    nc.scalar.activation(scr[:nlen, :NT], sp[:nlen, :NT], Exp)
                    nc.vector.tensor_reduce(zt[:nlen, bi, c:c + 1], scr[:nlen, :NT],
                                            axis=mybir.AxisListType.X,
                                            op=mybir.AluOpType.add)

            def emit_v(mt):
                m0, mlen = NBLK[mt]
                pp = psB.tile([P, 784], f32, tag="big", name="pp")
                for ko in range(KO):
                    for o0, w in ((0, 512), (512, 272)):
                        nc.tensor.matmul(pp[:mlen, o0:o0 + w], x_sb[:, ko, ds(m0, mlen)],
                                         wv[:, ko, ds(o0, w)], start=(ko == 0), stop=(ko == KO - 1))
                nc.vector.tensor_copy(v_sb[:mlen, mt, :], pp[:mlen, :])

            for t in range(8):
                emit_qkv(qt, 0, x_sb, t)
                emit_qkv(kt, 1, x_sb, t)
            for t in range(8):
                emit_qkv(qft, 0, xf_sb, t)
                emit_qkv(kft, 1, xf_sb, t)
                emit_pass1(2 * t)
                emit_pass1(2 * t + 1)
            for mt in range(5):
                emit_v(mt)
                for c in (16 + 3 * mt, 17 + 3 * mt, 18 + 3 * mt):
                    if c < 32:
                        emit_pass1(c)
            emit_pass1(31)

            # ---- -ln z, transposed to [c, n] ----
            negLnzT = qk.tile([32, NPAD], bf)
            for bi, (n0, nlen) in enumerate(NBLK):
                lnt = sc.tile([P, 32], bf, tag="lnt")
                nc.scalar.activation(lnt[:nlen, :], zt[:nlen, bi, :], Ln)
                ltp = psT.tile([32, P], bf, tag="tp")
                nc.tensor.transpose(ltp[:, :nlen], lnt[:nlen, :], idn[:nlen, :nlen])
                nc.vector.tensor_scalar_mul(negLnzT[:, ds(n0, nlen)], ltp[:, :nlen], -1.0)

            # scatter -ln z into the q augmentation rows (partition 48/112)
            for dst, c0 in ((qt, 0), (qft, 16)):
                for t in range(8):
                    nc.sync.dma_start(dst[48:49, t, :NT], negLnzT[c0 + 2 * t:c0 + 2 * t + 1, :NT])
                    nc.sync.dma_start(dst[112:113, t, :NT], negLnzT[c0 + 2 * t + 1:c0 + 2 * t + 2, :NT])

            # ---- E2T: per m-block: pass2 scores^T -> normalized A^T -> mix -> exp ----
            E2T = big.tile([P, 5, H, NPAD], bf, tag="E2T")
            for mi, (m0, mlen) in enumerate(NBLK):
                # A^T in slab-interleaved layout: [m, g, (c, nj)] where n = 4g + nj
                AT = wqkp.tile([P, NG, P], bf, tag="wqk")  # reuses wqk slot
                # pass 2: s^T - ln z via augmented K=49 contraction; exp -> A^T
                for c in range(32):
                    qs, ks = (qt, kt) if c < 16 else (qft, kft)
                    h = c % 16
                    t, off = h // 2, 64 * (h % 2)
                    sp = psB.tile([P, 784], f32, tag="big")
                    for o0, w in ((0, 512), (512, 65)):
                        nc.tensor.matmul(sp[:mlen, o0:o0 + w], ks[off:off + 49, t, ds(m0, mlen)],
                                         qs[off:off + 49, t, ds(o0, w)], start=True, stop=True)
                    nc.scalar.activation(AT[:mlen, :, ds(4 * c, 4)],
                                         sp[:mlen, :NPAD].rearrange("p (g m) -> p g m", m=4),
                                         Exp)
                # zero the n-padding lanes (n = 577..579 -> g = 144, nj = 1..3)
                nc.vector.memset(
                    AT[:mlen, NG - 1, :].rearrange("p (c m) -> p c m", m=4)[:, :, 1:4], 0.0)
                # mix: transpose 4-token slabs, matmul with conv block weights, exp.
                # Software-pipelined in batches of 4 groups; transposes land in pairs
                # in one PSUM tile so each PSUM->SBUF copy moves two slabs at once.
                # mix matmuls of batch b are issued after the transposes of batch b+1,
                # so the PE never waits on the copy.
                state = {"mp": None, "pend": []}

                def flush_exp2():
                    if not state["pend"]:
                        return
                    g0 = state["pend"][0][0]
                    tot = sum(p[1] for p in state["pend"])
                    nc.scalar.activation(
                        E2T[:mlen, mi, :, ds(4 * g0, 4 * tot)].rearrange("p o (g m) -> p g o m", g=tot),
                        state["mp"][:mlen, :tot, :].rearrange("p g (o m) -> p g o m", m=4),
                        Exp)
                    state["pend"] = []

                def emit_mix(pgb, png, prss):
                    if not state["pend"]:
                        state["mp"] = psM.tile([P, 8, 64], f32, tag="mp", name="mp")
                    off = sum(p[1] for p in state["pend"])
                    for gi in range(png):
                        nc.tensor.matmul(state["mp"][:mlen, off + gi, :],
                                         prss[gi // 2][:, gi % 2, :mlen], wblk[:],
                                         start=True, stop=True)
                    state["pend"].append((pgb, png))
                    if off + png >= 8:
                        flush_exp2()

                prev = None
                batches = [(gb, min(4, NG - gb)) for gb in range(0, NG, 4)]
                for b, (gb, ng) in enumerate(batches):
                    rss = []
                    for g2 in range(0, ng, 2):
                        n2 = min(2, ng - g2)
                        pool = psB if g2 % 4 == 0 else psT
                        ptag = "big" if g2 % 4 == 0 else "tp"
                        rp = pool.tile([P, 2, P], f32, tag=ptag)
                        for gi in range(g2, g2 + n2):
                            nc.tensor.matmul(rp[:, gi - g2, :mlen], AT[:mlen, gb + gi, :],
                                             idn[:mlen, :mlen], start=True, stop=True)
                        rs = rsp.tile([P, 2, P], bf, tag="rs")
                        nc.vector.tensor_copy(rs[:, :n2, :mlen], rp[:, :n2, :mlen])
                        rss.append(rs)
                    if prev is not None:
                        emit_mix(*prev)
                    prev = (gb, ng, rss)
                emit_mix(*prev)
                flush_exp2()

            # ---- PV: out[n, 48+1] per head, accumulate over m-blocks ----
            wt2 = qk.tile([P, H, HD], bf)
            wtT = qk.tile([P, KO, NPAD], bf)
            nc.vector.memset(wtT[:, 6, :], 0.0)
            nc.vector.memset(wtT[0:1, 6, :], 1.0)
            for ni, (n0, nlen) in enumerate(NBLK):
                for o in range(H):
                    pv = psM.tile([P, 64], f32, tag="mp")
                    for mi, (m0, mlen) in enumerate(NBLK):
                        nc.tensor.matmul(pv[:nlen, :49], E2T[:mlen, mi, o, ds(n0, nlen)],
                                         v_sb[:mlen, mi, ds(49 * o, 49)],
                                         start=(mi == 0), stop=(mi == 4))
                    zi2 = sc.tile([P, 1], f32, tag="zi")
                    nc.vector.reciprocal(zi2[:nlen], pv[:nlen, 48:49])
                    nc.vector.tensor_scalar_mul(wt2[:nlen, o, :], pv[:nlen, :48], zi2[:nlen])
                # transpose weighted [n, 768] -> [768, n] for the projection
                wt2f = wt2.rearrange("p o d -> p (o d)")
                for dt in range(6):
                    tp = psT.tile([P, P], f32, tag="tp")
                    nc.tensor.matmul(tp[:, :nlen], wt2f[:nlen, ts(dt, P)], idn[:nlen, :nlen],
                                     start=True, stop=True)
                    nc.scalar.copy(wtT[:, dt, ds(n0, nlen)], tp[:, :nlen])

            # ---- proj ----
            pwA = xw.tile([P, KO, 384], bf, tag="xw")  # reuses x slots
            pwB = xw.tile([P, KO, 384], bf, tag="xw")
            nc.sync.dma_start(pwA[:], pw_d[:, 0:384].rearrange("(ko p) m -> p ko m", p=P))
            nc.sync.dma_start(pwB[:], pw_d[:, 384:768].rearrange("(ko p) m -> p ko m", p=P))
            for dt in range(6):
                pw, dt_ = (pwA, dt) if dt < 3 else (pwB, dt - 3)
                fp = psB.tile([P, 784], f32, tag="big")
                for ko in range(KO):
                    for o0, w in ((0, 512), (512, 65)):
                        nc.tensor.matmul(fp[:, o0:o0 + w], pw[:, ko, ts(dt_, P)],
                                         wtT[:, ko, ds(o0, w)], start=(ko == 0), stop=(ko == KO - 1))
                ob = wqkp.tile([P, NT], f32, tag="wqk")
                nc.vector.tensor_copy(ob[:], fp[:, :NT])
                nc.sync.dma_start(out_d[ts(dt, P), :], ob[:])
    nc.finalize()
    return nc


def _prep_weights(qkv_w, qkv_b, conv_w, proj_w, proj_b):
    f = np.float32
    qkv_w, qkv_b = qkv_w.astype(f), qkv_b.astype(f)
    wqk = np.zeros((896, 2048), f)
    wv = np.zeros((896, 16 * 49), f)
    for h in range(H):
        q = slice(48 * h, 48 * h + 48)
        k = slice(768 + 48 * h, 768 + 48 * h + 48)
        v = slice(1536 + 48 * h, 1536 + 48 * h + 48)
        wqk[:768, 64 * h:64 * h + 48] = qkv_w[q, :].T * SCALE
        wqk[768, 64 * h:64 * h + 48] = qkv_b[q] * SCALE
        wqk[:768, 1024 + 64 * h:1024 + 64 * h + 48] = qkv_w[k, :].T
        wqk[768, 1024 + 64 * h:1024 + 64 * h + 48] = qkv_b[k]
        wqk[768, 1024 + 64 * h + 48] = 1.0          # k-tilde ones component
        wv[:768, 49 * h:49 * h + 48] = qkv_w[v, :].T
        wv[768, 49 * h:49 * h + 48] = qkv_b[v]
        wv[768, 49 * h + 48] = 1.0                  # softmax-2 denominator col
    wblk = np.zeros((128, 64), f)
    cw = conv_w.astype(f)
    for c in range(32):
        for nj in range(4):
            wblk[4 * c + nj, nj::4] = cw[:, c]
    pw = np.zeros((896, D), f)
    pw[:768, :] = proj_w.astype(f).T
    pw[768, :] = proj_b.astype(f)
    idn = np.eye(128, dtype=f)
    return {"wqkT": wqk.astype(BF), "wvT": wv.astype(BF), "wblk": wblk.astype(BF),
            "pwT": pw.astype(BF), "idn": idn.astype(BF)}


def kernel(x, x_freq, qkv_w, qkv_b, conv_w, conv_b, proj_w, proj_b, _profile=False):
    # conv_b is constant along the softmax axis -> cancels in softmax; unused.
    if "nc" not in _cache:
        _cache["nc"] = _build(bacc.Bacc())
    nc = _cache["nc"]
    wmap = _prep_weights(np.asarray(qkv_w), np.asarray(qkv_b), np.asarray(conv_w),
                         np.asarray(proj_w), np.asarray(proj_b))
    B = x.shape[0]
    in_maps = []
    for b in range(B):
        xT = np.zeros((896, NT), np.float32)
        xT[:768] = np.asarray(x[b], np.float32).T
        xT[768] = 1.0
        xfT = np.zeros((896, NT), np.float32)
        xfT[:768] = np.asarray(x_freq[b], np.float32).T
        xfT[768] = 1.0
        in_maps.append({"xT": xT.astype(BF), "xfT": xfT.astype(BF), **wmap})
    res = run_bass_kernel_spmd(nc, in_maps, core_ids=list(range(B)), trace=_profile)
    out = np.stack([res.results[b]["outT"].T for b in range(B)], axis=0)
    if _profile:
        return out.astype(np.float32), res
    return out.astype(np.float32)


# revision 38
# speedup vs baseline: 1.2504x; 1.2504x over previous
import sys
sys.path.insert(0, "/opt/trn_rl_repo")
import numpy as np
import ml_dtypes
import concourse.bacc as bacc
import concourse.bass as bass
import concourse.mybir as mybir
import concourse.tile as tile
from concourse.bass import ds, ts
from concourse.bass_utils import run_bass_kernel_spmd

BF = ml_dtypes.bfloat16
P = 128
NT = 577          # tokens
NPAD = 580        # tokens padded to 4*145
NG = 145          # token groups of 4 (for channel-mix transposes)
D = 768
H = 16
HD = 48
KO = 7            # 896 = 7*128 contraction tiles (768 dims + bias row + pad)
NBLK = [(0, 128), (128, 128), (256, 128), (384, 128), (512, 65)]
SCALE = HD ** -0.5

_cache = {}


def _build(nc):
    bf = mybir.dt.bfloat16
    f32 = mybir.dt.float32
    Exp = mybir.ActivationFunctionType.Exp
    Ln = mybir.ActivationFunctionType.Ln

    x_d = nc.dram_tensor("xT", [896, NT], bf, kind="ExternalInput")
    xf_d = nc.dram_tensor("xfT", [896, NT], bf, kind="ExternalInput")
    wqk_d = nc.dram_tensor("wqkT", [896, 2048], bf, kind="ExternalInput")
    wv_d = nc.dram_tensor("wvT", [896, 16 * 49], bf, kind="ExternalInput")
    wb_d = nc.dram_tensor("wblk", [P, 64], bf, kind="ExternalInput")
    id_d = nc.dram_tensor("idn", [P, P], bf, kind="ExternalInput")
    pw_d = nc.dram_tensor("pwT", [896, D], bf, kind="ExternalInput")
    out_d = nc.dram_tensor("outT", [D, NT], f32, kind="ExternalOutput")

    with tile.TileContext(nc) as tc:
        with tc.tile_pool(name="wqk", bufs=1) as wqkp, \
             tc.tile_pool(name="big", bufs=1) as big, \
             tc.tile_pool(name="xw", bufs=2) as xw, \
             tc.tile_pool(name="qk", bufs=1) as qk, \
             tc.tile_pool(name="sc", bufs=3) as sc, \
             tc.tile_pool(name="rsp", bufs=6) as rsp, \
             tc.tile_pool(name="psB", bufs=2, space="PSUM") as psB, \
             tc.tile_pool(name="psT", bufs=2, space="PSUM") as psT, \
             tc.tile_pool(name="psM", bufs=2, space="PSUM") as psM:

            # ---- load weights & inputs (q-half first so qkv can start early) ----
            wqk = wqkp.tile([P, KO, 2048], bf, tag="wqk")
            x_sb = xw.tile([P, KO, NPAD], bf, tag="xw")
            xf_sb = xw.tile([P, KO, NPAD], bf, tag="xw")
            nc.sync.dma_start(x_sb[:, :, :NT], x_d.rearrange("(ko p) m -> p ko m", p=P))
            nc.sync.dma_start(wqk[:, :, 0:1024], wqk_d[:, 0:1024].rearrange("(ko p) m -> p ko m", p=P))
            nc.sync.dma_start(wqk[:, :, 1024:2048], wqk_d[:, 1024:2048].rearrange("(ko p) m -> p ko m", p=P))
            nc.sync.dma_start(xf_sb[:, :, :NT], xf_d.rearrange("(ko p) m -> p ko m", p=P))
            wblk = qk.tile([P, 64], bf)
            idn = qk.tile([P, P], bf)
            nc.sync.dma_start(wblk[:], wb_d[:])
            nc.sync.dma_start(idn[:], id_d[:])
            wv = big.tile([P, KO, 784], bf, tag="E2T")  # slot reused by E2T later
            nc.sync.dma_start(wv[:], wv_d.rearrange("(ko p) m -> p ko m", p=P))

            # ---- qkv + pass 1 interleaved ----
            # qt/kt rows per head h: partitions 64*(h%2) .. +47 at t=h//2;
            # row 64*(h%2)+48 is the augmentation slot (ones for k, -ln z for q).
            # Pass-1 exp/reduce for the x-branch overlaps the PE-only qkv of the
            # freq branch; pass-1 of the freq branch overlaps the v matmuls.
            qt = qk.tile([P, 8, NT], bf)
            kt = qk.tile([P, 8, NT], bf)
            qft = qk.tile([P, 8, NT], bf)
            kft = qk.tile([P, 8, NT], bf)
            v_sb = qk.tile([P, 5, 784], bf)
            zt = qk.tile([P, 5, 32], f32)

            def emit_qkv(dst, wt_, src, t):
                pp = psB.tile([P, 784], f32, tag="big", name="pp")
                for ko in range(KO):
                    for o0, w in ((0, 512), (512, 65)):
                        nc.tensor.matmul(pp[:, o0:o0 + w], wqk[:, ko, ds(1024 * wt_ + 128 * t, P)],
                                         src[:, ko, ds(o0, w)],
                                         start=(ko == 0), stop=(ko == KO - 1))
                nc.vector.tensor_copy(dst[:, t, :NT], pp[:, :NT])

            def emit_pass1(c):
                # processes channel pair (c, c+1); called with even c.
                # the two exps share one scratch tile and one paired DVE reduce.
                for bi, (n0, nlen) in enumerate(NBLK):
                    scr = sc.tile([P, 2, NPAD], bf, tag="scr", name="scr", bufs=2)
                    for ci in (c, c + 1):
                        qs, ks = (qt, kt) if ci < 16 else (qft, kft)
                        h = ci % 16
                        t, off = h // 2, 64 * (h % 2)
                        sp = psB.tile([P, 784], f32, tag="big", name="sp")
                        for o0, w in ((0, 512), (512, 65)):
                            nc.tensor.matmul(sp[:nlen, o0:o0 + w], qs[off:off + 48, t, ds(n0, nlen)],
                                             ks[off:off + 48, t, ds(o0, w)], start=True, stop=True)
                        nc.scalar.activation(scr[:nlen, ci - c, :NT], sp[:nlen, :NT], Exp)
                    nc.vector.tensor_reduce(zt[:nlen, bi, c:c + 2],
                                            scr[:nlen, :, :NT],
                                            axis=mybir.AxisListType.X,
                                            op=mybir.AluOpType.add)

            def emit_v(mt):
                m0, mlen = NBLK[mt]
                pp = psB.tile([P, 784], f32, tag="big", name="pp")
                for ko in range(KO):
                    for o0, w in ((0, 512), (512, 272)):
                        nc.tensor.matmul(pp[:mlen, o0:o0 + w], x_sb[:, ko, ds(m0, mlen)],
                                         wv[:, ko, ds(o0, w)], start=(ko == 0), stop=(ko == KO - 1))
                nc.vector.tensor_copy(v_sb[:mlen, mt, :], pp[:mlen, :])

            for t in range(8):
                emit_qkv(qt, 0, x_sb, t)
                emit_qkv(kt, 1, x_sb, t)
            for t in range(8):
                emit_qkv(qft, 0, xf_sb, t)
                emit_qkv(kft, 1, xf_sb, t)
            for c in range(0, 32, 2):
                emit_pass1(c)
            for mt in range(5):
                emit_v(mt)

            # ---- -ln z, transposed to [c, n] ----
            negLnzT = qk.tile([32, NPAD], bf)
            for bi, (n0, nlen) in enumerate(NBLK):
                lnt = sc.tile([P, 32], bf, tag="lnt")
                nc.scalar.activation(lnt[:nlen, :], zt[:nlen, bi, :], Ln)
                ltp = psT.tile([32, P], bf, tag="tp")
                nc.tensor.transpose(ltp[:, :nlen], lnt[:nlen, :], idn[:nlen, :nlen])
                nc.vector.tensor_scalar_mul(negLnzT[:, ds(n0, nlen)], ltp[:, :nlen], -1.0)

            # scatter -ln z into the q augmentation rows (partition 48/112)
            for dst, c0 in ((qt, 0), (qft, 16)):
                for t in range(8):
                    nc.sync.dma_start(dst[48:49, t, :NT], negLnzT[c0 + 2 * t:c0 + 2 * t + 1, :NT])
                    nc.sync.dma_start(dst[112:113, t, :NT], negLnzT[c0 + 2 * t + 1:c0 + 2 * t + 2, :NT])

            # ---- E2T: per m-block: pass2 scores^T -> normalized A^T -> mix -> exp ----
            E2T = big.tile([P, 5, H, NPAD], bf, tag="E2T")
            for mi, (m0, mlen) in enumerate(NBLK):
                # A^T in slab-interleaved layout: [m, g, (c, nj)] where n = 4g + nj
                AT = wqkp.tile([P, NG, P], bf, tag="wqk")  # reuses wqk slot
                # pass 2: s^T - ln z via augmented K=49 contraction; exp -> A^T
                for c in range(32):
                    qs, ks = (qt, kt) if c < 16 else (qft, kft)
                    h = c % 16
                    t, off = h // 2, 64 * (h % 2)
                    sp = psB.tile([P, 784], f32, tag="big")
                    for o0, w in ((0, 512), (512, 65)):
                        nc.tensor.matmul(sp[:mlen, o0:o0 + w], ks[off:off + 49, t, ds(m0, mlen)],
                                         qs[off:off + 49, t, ds(o0, w)], start=True, stop=True)
                    nc.scalar.activation(AT[:mlen, :, ds(4 * c, 4)],
                                         sp[:mlen, :NPAD].rearrange("p (g m) -> p g m", m=4),
                                         Exp)
                # zero the n-padding lanes (n = 577..579 -> g = 144, nj = 1..3)
                nc.vector.memset(
                    AT[:mlen, NG - 1, :].rearrange("p (c m) -> p c m", m=4)[:, :, 1:4], 0.0)
                # mix: transpose 4-token slabs, matmul with conv block weights, exp.
                # Software-pipelined in batches of 4 groups; transposes land in pairs
                # in one PSUM tile so each PSUM->SBUF copy moves two slabs at once.
                # mix matmuls of batch b are issued after the transposes of batch b+1,
                # so the PE never waits on the copy.
                state = {"mp": None, "pend": []}

                def flush_exp2():
                    if not state["pend"]:
                        return
                    g0 = state["pend"][0][0]
                    tot = sum(p[1] for p in state["pend"])
                    nc.scalar.activation(
                        E2T[:mlen, mi, :, ds(4 * g0, 4 * tot)].rearrange("p o (g m) -> p g o m", g=tot),
                        state["mp"][:mlen, :tot, :].rearrange("p g (o m) -> p g o m", m=4),
                        Exp)
                    state["pend"] = []

                def emit_mix(pgb, png, prss):
                    if not state["pend"]:
                        state["mp"] = psM.tile([P, 8, 64], f32, tag="mp", name="mp")
                    off = sum(p[1] for p in state["pend"])
                    for gi in range(png):
                        nc.tensor.matmul(state["mp"][:mlen, off + gi, :],
                                         prss[gi // 2][:, gi % 2, :mlen], wblk[:],
                                         start=True, stop=True)
                    state["pend"].append((pgb, png))
                    if off + png >= 8:
                        flush_exp2()

                prev = None
                batches = [(gb, min(4, NG - gb)) for gb in range(0, NG, 4)]
                for b, (gb, ng) in enumerate(batches):
                    rss = []
                    for g2 in range(0, ng, 2):
                        n2 = min(2, ng - g2)
                        pool = psB if g2 % 4 == 0 else psT
                        ptag = "big" if g2 % 4 == 0 else "tp"
                        rp = pool.tile([P, 2, P], f32, tag=ptag)
                        for gi in range(g2, g2 + n2):
                            nc.tensor.matmul(rp[:, gi - g2, :mlen], AT[:mlen, gb + gi, :],
                                             idn[:mlen, :mlen], start=True, stop=True)
                        rs = rsp.tile([P, 2, P], bf, tag="rs")
                        nc.vector.tensor_copy(rs[:, :n2, :mlen], rp[:, :n2, :mlen])
                        rss.append(rs)
                    if prev is not None:
                        emit_mix(*prev)
                    prev = (gb, ng, rss)
                emit_mix(*prev)
                flush_exp2()

            # ---- PV: out[n, 48+1] per head, accumulate over m-blocks ----
            wt2 = qk.tile([P, H, HD], bf)
            wtT = qk.tile([P, KO, NPAD], bf)
            nc.vector.memset(wtT[:, 6, :], 0.0)
            nc.vector.memset(wtT[0:1, 6, :], 1.0)
            for ni, (n0, nlen) in enumerate(NBLK):
                for o in range(H):
                    pv = psM.tile([P, 64], f32, tag="mp")
                    for mi, (m0, mlen) in enumerate(NBLK):
                        nc.tensor.matmul(pv[:nlen, :49], E2T[:mlen, mi, o, ds(n0, nlen)],
                                         v_sb[:mlen, mi, ds(49 * o, 49)],
                                         start=(mi == 0), stop=(mi == 4))
                    zi2 = sc.tile([P, 1], f32, tag="zi")
                    nc.vector.reciprocal(zi2[:nlen], pv[:nlen, 48:49])
                    nc.vector.tensor_scalar_mul(wt2[:nlen, o, :], pv[:nlen, :48], zi2[:nlen])
                # transpose weighted [n, 768] -> [768, n] for the projection
                wt2f = wt2.rearrange("p o d -> p (o d)")
                for dt in range(6):
                    tp = psT.tile([P, P], f32, tag="tp")
                    nc.tensor.matmul(tp[:, :nlen], wt2f[:nlen, ts(dt, P)], idn[:nlen, :nlen],
                                     start=True, stop=True)
                    nc.scalar.copy(wtT[:, dt, ds(n0, nlen)], tp[:, :nlen])

            # ---- proj ----
            pwA = xw.tile([P, KO, 384], bf, tag="xw")  # reuses x slots
            pwB = xw.tile([P, KO, 384], bf, tag="xw")
            nc.sync.dma_start(pwA[:], pw_d[:, 0:384].rearrange("(ko p) m -> p ko m", p=P))
            nc.sync.dma_start(pwB[:], pw_d[:, 384:768].rearrange("(ko p) m -> p ko m", p=P))
            for dt in range(6):
                pw, dt_ = (pwA, dt) if dt < 3 else (pwB, dt - 3)
                fp = psB.tile([P, 784], f32, tag="big")
                for ko in range(KO):
                    for o0, w in ((0, 512), (512, 65)):
                        nc.tensor.matmul(fp[:, o0:o0 + w], pw[:, ko, ts(dt_, P)],
                                         wtT[:, ko, ds(o0, w)], start=(ko == 0), stop=(ko == KO - 1))
                ob = wqkp.tile([P, NT], f32, tag="wqk")
                nc.vector.tensor_copy(ob[:], fp[:, :NT])
                nc.sync.dma_start(out_d[ts(dt, P), :], ob[:])
    nc.finalize()
    return nc


def _prep_weights(qkv_w, qkv_b, conv_w, proj_w, proj_b):
    f = np.float32
    qkv_w, qkv_b = qkv_w.astype(f), qkv_b.astype(f)
    wqk = np.zeros((896, 2048), f)
    wv = np.zeros((896, 16 * 49), f)
    for h in range(H):
        q = slice(48 * h, 48 * h + 48)
        k = slice(768 + 48 * h, 768 + 48 * h + 48)
        v = slice(1536 + 48 * h, 1536 + 48 * h + 48)
        wqk[:768, 64 * h:64 * h + 48] = qkv_w[q, :].T * SCALE
        wqk[768, 64 * h:64 * h + 48] = qkv_b[q] * SCALE
        wqk[:768, 1024 + 64 * h:1024 + 64 * h + 48] = qkv_w[k, :].T
        wqk[768, 1024 + 64 * h:1024 + 64 * h + 48] = qkv_b[k]
        wqk[768, 1024 + 64 * h + 48] = 1.0          # k-tilde ones component
        wv[:768, 49 * h:49 * h + 48] = qkv_w[v, :].T
        wv[768, 49 * h:49 * h + 48] = qkv_b[v]
        wv[768, 49 * h + 48] = 1.0                  # softmax-2 denominator col
    wblk = np.zeros((128, 64), f)
    cw = conv_w.astype(f)
    for c in range(32):
        for nj in range(4):
            wblk[4 * c + nj, nj::4] = cw[:, c]
    pw = np.zeros((896, D), f)
    pw[:768, :] = proj_w.astype(f).T
    pw[768, :] = proj_b.astype(f)
    idn = np.eye(128, dtype=f)
    return {"wqkT": wqk.astype(BF), "wvT": wv.astype(BF), "wblk": wblk.astype(BF),
            "pwT": pw.astype(BF), "idn": idn.astype(BF)}


def kernel(x, x_freq, qkv_w, qkv_b, conv_w, conv_b, proj_w, proj_b, _profile=False):
    # conv_b is constant along the softmax axis -> cancels in softmax; unused.
    if "nc" not in _cache:
        _cache["nc"] = _build(bacc.Bacc())
    nc = _cache["nc"]
    wmap = _prep_weights(np.asarray(qkv_w), np.asarray(qkv_b), np.asarray(conv_w),
                         np.asarray(proj_w), np.asarray(proj_b))
    B = x.shape[0]
    in_maps = []
    for b in range(B):
        xT = np.zeros((896, NT), np.float32)
        xT[:768] = np.asarray(x[b], np.float32).T
        xT[768] = 1.0
        xfT = np.zeros((896, NT), np.float32)
        xfT[:768] = np.asarray(x_freq[b], np.float32).T
        xfT[768] = 1.0
        in_maps.append({"xT": xT.astype(BF), "xfT": xfT.astype(BF), **wmap})
    res = run_bass_kernel_spmd(nc, in_maps, core_ids=list(range(B)), trace=_profile)
    out = np.stack([res.results[b]["outT"].T for b in range(B)], axis=0)
    if _profile:
        return out.astype(np.float32), res
    return out.astype(np.float32)
